# revision 18
# baseline (speedup 1.0000x reference)
"""Trainium2 Bass kernel for nn_DecoderLayer_70205535421363.

Decoder layer (pre-LN, T5-style RMSNorm, QK-norm attention + gated-silu MLP)
B=2, S=2048, D=2048, H=16, HD=128, F=8192, fp32.

Strategy: 8 cores = 2 batches x 4 shards, two SPMD launches.
  Launch A (attention): core c handles batch c//4, heads 4*(c%4)..+4.
    Q/K projections + scores in float32r (bf16-rate matmul, ~tf32 precision);
    unnormalized softmax (exp on ACT, denominators via a ones-column in the
    AV matmul); AV + output projection in bf16. Outputs per-core partial
    attention output; host sums the 4 head-shards per batch.
  Launch B (MLP): core c handles batch c//4, mlp columns 2048*(c%4)..+2048.
    Host computes inter = x + attn_out and pre-norms/transposes it; device
    does the three matmuls + silu gating in bf16, fp32 accumulation.
All learned norm scales are folded on the host (ln1/ln2 into the weights;
qln*kln applied to q_hat on device; rmsnorm r factors cancel for q/k and are
applied to v / folded into h on host).
"""
import os

import numpy as np
import ml_dtypes
from contextlib import ExitStack

import jax
from jax.sharding import Mesh, PartitionSpec, NamedSharding
from jax.experimental.shard_map import shard_map

import concourse.bass as bass
import concourse.tile as tile
import concourse.mybir as mybir
from concourse import bass2jax
from concourse.bass2jax import _bass_exec_p, install_neuronx_cc_hook, partition_id_tensor
from concourse.vector_clock import ScopedClock
from concourse.masks import make_identity

F32 = mybir.dt.float32
F32R = mybir.dt.float32r
F16 = mybir.dt.float16
BF16 = mybir.dt.bfloat16
AF = mybir.ActivationFunctionType
bf16 = ml_dtypes.bfloat16
f16 = np.float16

B, S, D, H, HD, F = 2, 2048, 2048, 16, 128, 8192
EPS = 1e-6
NH = 4            # heads per core
FL = F // 4       # mlp columns per core
ST = S // 128
DT = D // 128
FT = FL // 128
N_CORES = 8

MAX_WAITS = 1     # this walrus build allows one sync-wait per instruction


# ---------------------------------------------------------------------------
# Tile workarounds for the 1-sync-wait-per-instruction walrus limit
# ---------------------------------------------------------------------------
class TileContextFixed(tile.TileContext):
    def _drain_and_barrier(self, tick_clock, wait_clock):
        nc = self.nc
        probe = nc.sync.nop(nofuse=True)
        wait_clock.add_sem_waits(probe.ins, ScopedClock({None: tick_clock.global_clock}))
        si = probe.ins.sync_info
        waits = list(si.on_wait) if si is not None else []
        if len(waits) > MAX_WAITS:
            si.on_wait = waits[:MAX_WAITS]
            rest = waits[MAX_WAITS:]
            for i in range(0, len(rest), MAX_WAITS):
                extra = nc.sync.nop(nofuse=True)
                extra.ins.sync_info = mybir.SyncInfo(
                    on_wait=rest[i:i + MAX_WAITS], on_update=[])
        nc.sync.drain()
        nc.all_engine_barrier()
        assert self.sems is not None
        popped = nc._tile_sem_poison_stack.pop()
        assert popped is self._sem_poison
        nc.clear_and_free_semaphores(list(self.sems.allocated().values()))
        nc.all_engine_barrier()


def legalize_waits(nc, max_waits=MAX_WAITS):
    for fn in nc.m.functions:
        for bb in fn.blocks:
            insts = bb.instructions
            new_insts = []
            changed = False
            for inst in insts:
                si = inst.sync_info
                if si is not None and len(si.on_wait) > max_waits:
                    waits = list(si.on_wait)
                    keep = waits[:max_waits]
                    rest = waits[max_waits:]
                    for i in range(0, len(rest), max_waits):
                        nop = mybir.InstNoOp(
                            name=nc.get_next_instruction_name(),
                            engine=inst.engine, ins=[], outs=[])
                        nop.sync_info = mybir.SyncInfo(
                            on_wait=rest[i:i + max_waits], on_update=[])
                        nc.register_instruction(nop)
                        new_insts.append(nop)
                        changed = True
                    si.on_wait = keep
                new_insts.append(inst)
            if changed:
                insts.clear()
                insts.extend(new_insts)


# ---------------------------------------------------------------------------
# Launch A: attention partial (one batch, NH heads per core)
# ---------------------------------------------------------------------------
def build_attn(reps=1):
    nc = bass.Bass()
    xt = nc.dram_tensor("xt", [D, S], F16, kind="ExternalInput")
    rv = nc.dram_tensor("rv", [128, ST], F32, kind="ExternalInput")
    wq = nc.dram_tensor("wq", [D, NH * HD], F16, kind="ExternalInput")
    wk = nc.dram_tensor("wk", [D, NH * HD], F16, kind="ExternalInput")
    wv = nc.dram_tensor("wv", [D, NH * HD], F16, kind="ExternalInput")
    wo = nc.dram_tensor("wo", [NH * HD, D], BF16, kind="ExternalInput")
    qw = nc.dram_tensor("qw", [128, NH * HD], F16, kind="ExternalInput")
    maska = nc.dram_tensor("maska", [128, 1024], BF16, kind="ExternalInput")
    maskb = nc.dram_tensor("maskb", [128, 1024], BF16, kind="ExternalInput")
    attn = nc.dram_tensor("attn", [S, D], F32, kind="ExternalOutput")

    # p-major views for single-DMA loads
    xt_p = xt.rearrange("(dt p) s -> p dt s", p=128)
    wq_p = wq.rearrange("(dt p) f -> p dt f", p=128)
    wk_p = wk.rearrange("(dt p) f -> p dt f", p=128)
    wv_bp = wv.rearrange("(dt p) f -> p dt f", p=128)
    wo_p = wo.rearrange("(h p) d -> p h d", p=128)

    with TileContextFixed(nc) as tc:
      for _rep in range(reps):
       with ExitStack() as top:
        consts = top.enter_context(tc.tile_pool(name="consts", bufs=1))
        qw_sb = consts.tile([128, NH * HD], F16, name="qw_sb")
        maska_sb = consts.tile([128, 1024], BF16, name="maska_sb")
        maskb_sb = consts.tile([128, 1024], BF16, name="maskb_sb")
        rv_sb = consts.tile([128, ST], F32, name="rv_sb")
        eps_sb = consts.tile([128, 1], F32, name="eps_sb")
        nc.vector.memset(eps_sb, EPS)
        id_h = consts.tile([128, 128], F16, name="id_h")
        make_identity(nc, id_h)
        id_b = consts.tile([128, 128], BF16, name="id_b")
        make_identity(nc, id_b)
        # wo prefetched at top level so phase 2 starts without a DMA stall
        wopool = top.enter_context(tc.tile_pool(name="wop", bufs=1))
        wo_sb = wopool.tile([128, NH, D], BF16, name="wo_sb")

        persist = top.enter_context(tc.tile_pool(name="persist", bufs=1))
        QT = [persist.tile([128, S], F16, tag=f"qt{h}", name=f"qt{h}")
              for h in range(NH)]
        KT = [persist.tile([128, S], F16, tag=f"kt{h}", name=f"kt{h}")
              for h in range(NH)]
        VA = [persist.tile([128, ST, 132], BF16, tag=f"va{h}", name=f"va{h}")
              for h in range(NH)]
        for h in range(NH):
            nc.vector.memset(VA[h][:, :, 128:129], 1.0)

        # ---- phase 1a: Q, K projections + per-head rmsnorm + transpose ----
        with ExitStack() as ph:
            wpool = ph.enter_context(tc.tile_pool(name="wqk", bufs=1))
            xpool = ph.enter_context(tc.tile_pool(name="xcol", bufs=2))
            wq_sb = [wpool.tile([128, 4, NH * HD], F16, name=f"wq_g{g}")
                     for g in range(4)]
            wk_sb = [wpool.tile([128, 4, NH * HD], F16, name=f"wk_g{g}")
                     for g in range(4)]
            wv_bf = [wpool.tile([128, 4, NH * HD], F16, name=f"wv_g{g}")
                     for g in range(4)]
            xq = []

            def load_x(st):
                xcol = xpool.tile([128, DT, 128], F16, tag="x", name="xcol")
                nc.sync.dma_start(out=xcol,
                                  in_=xt_p[:, :, st * 128:(st + 1) * 128])
                xq.append(xcol)

            # x strip + wq first (first matmuls depend only on these), then
            # wk/wv, then the small consts and the phase-2 wo prefetch.
            load_x(0)
            for g in range(4):
                nc.sync.dma_start(out=wq_sb[g], in_=wq_p[:, 4 * g:4 * g + 4, :])
                if g == 0:
                    load_x(1)
            for g in range(4):
                nc.sync.dma_start(out=wk_sb[g], in_=wk_p[:, 4 * g:4 * g + 4, :])
                nc.sync.dma_start(out=wv_bf[g], in_=wv_bp[:, 4 * g:4 * g + 4, :])
            nc.sync.dma_start(out=rv_sb, in_=rv[:, :])
            nc.sync.dma_start(out=qw_sb, in_=qw[:, :])
            nc.sync.dma_start(out=maska_sb, in_=maska[:, :])
            nc.sync.dma_start(out=maskb_sb, in_=maskb[:, :])
            for h in range(NH):
                nc.sync.dma_start(out=wo_sb[:, h, :], in_=wo_p[:, h, :])
            tpool = ph.enter_context(tc.tile_pool(name="p1tmp", bufs=2))
            spool = ph.enter_context(tc.tile_pool(name="p1sc", bufs=3))
            psq = ph.enter_context(tc.tile_pool(name="psq", bufs=2, space="PSUM"))
            pst = ph.enter_context(tc.tile_pool(name="pst", bufs=1, space="PSUM"))

            def p1_mm_norm(st):
                if st + 2 < ST:
                    load_x(st + 2)
                xcol = xq.pop(0)
                q_ps = psq.tile([128, NH * HD], F32, tag="q", name="q_ps")
                k_ps = psq.tile([128, NH * HD], F32, tag="k", name="k_ps")
                for d in range(DT):
                    nc.tensor.matmul(q_ps, xcol[:, d, :], wq_sb[d // 4][:, d % 4, :],
                                     start=(d == 0), stop=(d == DT - 1))
                for d in range(DT):
                    nc.tensor.matmul(k_ps, xcol[:, d, :], wk_sb[d // 4][:, d % 4, :],
                                     start=(d == 0), stop=(d == DT - 1))
                v_ps = psq.tile([128, NH * HD], F32, tag="v", name="v_ps")
                for d in range(DT):
                    nc.tensor.matmul(v_ps, xcol[:, d, :], wv_bf[d // 4][:, d % 4, :],
                                     start=(d == 0), stop=(d == DT - 1))
                for h in range(NH):
                    sl = slice(h * HD, (h + 1) * HD)
                    nc.scalar.activation(out=VA[h][:, st, 0:128], in_=v_ps[:, sl],
                                         func=AF.Copy, scale=rv_sb[:, st:st + 1])
                sq = spool.tile([128, HD], F32, tag="sq", name="sq")
                ssq = spool.tile([128, NH], F32, tag="ssq", name="ssq")
                ssk = spool.tile([128, NH], F32, tag="ssk", name="ssk")
                for h in range(NH):
                    sl = slice(h * HD, (h + 1) * HD)
                    nc.scalar.activation(out=sq, in_=q_ps[:, sl],
                                         func=AF.Square, accum_out=ssq[:, h:h + 1])
                for h in range(NH):
                    sl = slice(h * HD, (h + 1) * HD)
                    nc.scalar.activation(out=sq, in_=k_ps[:, sl],
                                         func=AF.Square, accum_out=ssk[:, h:h + 1])
                rq = spool.tile([128, NH], F32, tag="rq", name="rq")
                nc.scalar.activation(out=rq, in_=ssq, func=AF.Sqrt,
                                     scale=1.0 / HD, bias=eps_sb)
                nc.vector.reciprocal(rq, rq)
                rk = spool.tile([128, NH], F32, tag="rk", name="rk")
                nc.scalar.activation(out=rk, in_=ssk, func=AF.Sqrt,
                                     scale=1.0 / HD, bias=eps_sb)
                nc.vector.reciprocal(rk, rk)
                qh = tpool.tile([128, NH * HD], F16, tag="qh", name="qh")
                for h in range(NH):
                    sl = slice(h * HD, (h + 1) * HD)
                    nc.vector.tensor_scalar_mul(qh[:, sl], q_ps[:, sl], rq[:, h:h + 1])
                nc.vector.tensor_tensor(out=qh, in0=qh, in1=qw_sb,
                                        op=mybir.AluOpType.mult)
                ksb = tpool.tile([128, NH * HD], F16, tag="ksb", name="ksb")
                for h in range(NH):
                    sl = slice(h * HD, (h + 1) * HD)
                    nc.vector.tensor_scalar_mul(ksb[:, sl], k_ps[:, sl], rk[:, h:h + 1])
                return qh, ksb

            def p1_transpose(st, qh, ksb):
                qtk = pst.tile([128, 2 * NH * HD], F16, tag="qtk", name="qtk")
                qt_ps = qtk[:, 0:NH * HD]
                kt_ps = qtk[:, NH * HD:2 * NH * HD]
                for h in range(NH):
                    sl = slice(h * HD, (h + 1) * HD)
                    nc.tensor.transpose(qt_ps[:, sl], qh[:, sl], id_h)
                    nc.tensor.transpose(kt_ps[:, sl], ksb[:, sl], id_h)
                ssl = slice(st * 128, (st + 1) * 128)
                for h in range(NH):
                    sl = slice(h * HD, (h + 1) * HD)
                    nc.vector.tensor_copy(QT[h][:, ssl], qt_ps[:, sl])
                    nc.vector.tensor_copy(KT[h][:, ssl], kt_ps[:, sl])

            prev = None
            for st in range(ST):
                cur = p1_mm_norm(st)
                if prev is not None:
                    p1_transpose(prev[0], *prev[1])
                prev = (st, cur)
            p1_transpose(prev[0], *prev[1])

        # ---- phase 2+3: attention + output projection, qc-outer ----
        with ExitStack() as ph:
            ppool = ph.enter_context(tc.tile_pool(name="probs", bufs=6))
            fpool = ph.enter_context(tc.tile_pool(name="fin", bufs=4))
            apool = ph.enter_context(tc.tile_pool(name="attnT", bufs=2))
            opool = ph.enter_context(tc.tile_pool(name="outp", bufs=4))
            pslg = ph.enter_context(tc.tile_pool(name="pslg", bufs=2, space="PSUM"))
            psav = ph.enter_context(tc.tile_pool(name="psav", bufs=1, space="PSUM"))
            psat = ph.enter_context(tc.tile_pool(name="psat", bufs=1, space="PSUM"))
            psot = ph.enter_context(tc.tile_pool(name="psot", bufs=1, space="PSUM"))

            def attend_head(h, qc, attnT_h):
                q0 = qc * 512
                att2 = [psav.tile([128, 2, 132], F32, tag=f"att{i}",
                                  name=f"att{i}") for i in range(2)]
                att_ps = [att2[qs // 2][:, qs % 2, :] for qs in range(4)]
                for p in range(2 * qc + 2):
                    lg = pslg.tile([128, 1024], F32, tag="lg", name="lg")
                    for half in range(2):
                        kt = 2 * p + half
                        nc.tensor.matmul(
                            lg[:, half * 512:(half + 1) * 512],
                            KT[h][:, kt * 128:(kt + 1) * 128],
                            QT[h][:, q0:q0 + 512], start=True, stop=True)
                    pr = ppool.tile([128, 1024], BF16, tag="pr", name="pr")
                    nc.scalar.activation(out=pr, in_=lg, func=AF.Exp)
                    if 2 * p >= qc * 4:
                        msk = maska_sb if 2 * p == qc * 4 else maskb_sb
                        nc.vector.tensor_tensor(
                            out=pr, in0=pr, in1=msk, op=mybir.AluOpType.mult)
                    for half in range(2):
                        kt = 2 * p + half
                        for qs in range(4):
                            qt = qc * 4 + qs
                            if kt <= qt:
                                # two accumulators share a PSUM bank; start=True
                                # clears has_written bank-wide, so only the first
                                # group in each bank may use it — the second
                                # group's kt=0 write lands on cleared bits and
                                # overwrites rather than accumulates.
                                nc.tensor.matmul(
                                    att_ps[qs][:, 0:129],
                                    pr[:, half * 512 + qs * 128:
                                       half * 512 + (qs + 1) * 128],
                                    VA[h][:, kt, 0:129],
                                    start=(kt == 0 and qs % 2 == 0),
                                    stop=(kt == qt), skip_group_check=True)
                return att_ps

            def finalize_head(qc, att_ps, attnT_h):
                for qs in range(4):
                    dr = fpool.tile([128, 1], F32, tag="dr", name="dr")
                    nc.vector.reciprocal(dr, att_ps[qs][:, 128:129])
                    asb = fpool.tile([128, 128], BF16, tag="asb", name="asb")
                    nc.vector.tensor_scalar_mul(asb, att_ps[qs][:, 0:128], dr)
                    at_ps = psat.tile([128, 128], BF16, tag="atp", name="at_ps")
                    nc.tensor.transpose(at_ps, asb, id_b)
                    nc.scalar.activation(
                        out=attnT_h[:, qs * 128:(qs + 1) * 128], in_=at_ps,
                        func=AF.Copy)

            def emit_wo(attnT, qc):
                for qs in range(4):
                    qt = qc * 4 + qs
                    for dc in range(D // 512):
                        o_ps = psot.tile([128, 512], F32, tag="o", name="o_ps")
                        for h in range(NH):
                            nc.tensor.matmul(
                                o_ps, attnT[h][:, qs * 128:(qs + 1) * 128],
                                wo_sb[:, h, dc * 512:(dc + 1) * 512],
                                start=(h == 0), stop=(h == NH - 1))
                        osb = opool.tile([128, 512], F32, tag="osb", name="osb")
                        nc.vector.tensor_copy(osb, o_ps)
                        nc.sync.dma_start(
                            out=attn[qt * 128:(qt + 1) * 128,
                                     dc * 512:(dc + 1) * 512],
                            in_=osb)

            pending = None
            for qc in range(S // 512):
                attnT = [apool.tile([128, 512], BF16, tag=f"at{h}", name=f"at{h}")
                         for h in range(NH)]
                prev = None
                for h in range(NH):
                    att_ps = attend_head(h, qc, attnT[h])
                    if h == 0 and pending is not None:
                        emit_wo(*pending)
                        pending = None
                    if prev is not None:
                        finalize_head(qc, prev[1], attnT[prev[0]])
                    prev = (h, att_ps)
                finalize_head(qc, prev[1], attnT[prev[0]])
                pending = (attnT, qc)
            emit_wo(*pending)
    legalize_waits(nc)
    return nc


# ---------------------------------------------------------------------------
# Launch B: gated MLP partial (one batch, FL mlp columns per core)
# ---------------------------------------------------------------------------
def build_mlp(reps=1):
    nc = bass.Bass()
    ht = nc.dram_tensor("ht", [D, S], BF16, kind="ExternalInput")
    wi0 = nc.dram_tensor("wi0", [D, FL], BF16, kind="ExternalInput")
    wi1 = nc.dram_tensor("wi1", [D, FL], BF16, kind="ExternalInput")
    wog = nc.dram_tensor("wog", [FL, D], BF16, kind="ExternalInput")
    mlp = nc.dram_tensor("mlp", [S, D], F32, kind="ExternalOutput")

    wi0_p = wi0.rearrange("(dt p) f -> p dt f", p=128)
    wi1_p = wi1.rearrange("(dt p) f -> p dt f", p=128)
    ht_p = ht.rearrange("(dt p) s -> p dt s", p=128)
    wog_p = wog.rearrange("(ft p) d -> p ft d", p=128)

    with TileContextFixed(nc) as tc:
      for _rep in range(reps):
       with ExitStack() as top:
        persist = top.enter_context(tc.tile_pool(name="persist", bufs=1))
        gT = [persist.tile([128, S], BF16, tag=f"g{f}", name=f"g{f}")
              for f in range(FT)]
        # first wog group prefetched during phase 1 so phase 2 starts hot
        wog0 = persist.tile([128, 4, D], BF16, tag="wog0", name="wog0")

        with ExitStack() as ph:
            hpool = ph.enter_context(tc.tile_pool(name="htp", bufs=1))
            wpool = ph.enter_context(tc.tile_pool(name="wcol", bufs=3))
            gpool = ph.enter_context(tc.tile_pool(name="gtmp", bufs=4))
            psab = ph.enter_context(tc.tile_pool(name="psab", bufs=2, space="PSUM"))

            def load_w(ft):
                fsl = slice(ft * 128, (ft + 1) * 128)
                w0c = wpool.tile([128, DT, 128], BF16, tag="w0", name="w0c")
                w1c = wpool.tile([128, DT, 128], BF16, tag="w1", name="w1c")
                nc.sync.dma_start(out=w0c, in_=wi0_p[:, :, fsl])
                nc.sync.dma_start(out=w1c, in_=wi1_p[:, :, fsl])
                return w0c, w1c

            wq_pending = [load_w(0)]
            hT = [hpool.tile([128, S], BF16, tag=f"h{d}", name=f"h{d}")
                  for d in range(DT)]
            # first s-wave of hT lands first so matmuls start early
            for d in range(DT):
                nc.sync.dma_start(out=hT[d][:, 0:512], in_=ht_p[:, d, 0:512])
            for d in range(DT):
                nc.sync.dma_start(out=hT[d][:, 512:S], in_=ht_p[:, d, 512:S])
            for f in range(4):
                nc.sync.dma_start(out=wog0[:, f, :], in_=wog_p[:, f, :])
            for ft in range(FT):
                if ft + 1 < FT:
                    wq_pending.append(load_w(ft + 1))
                w0c, w1c = wq_pending.pop(0)
                for sc in range(S // 512):
                    ssl = slice(sc * 512, (sc + 1) * 512)
                    a_ps = psab.tile([128, 512], F32, tag="a", name="a_ps")
                    b_ps = psab.tile([128, 512], F32, tag="b", name="b_ps")
                    for d in range(DT):
                        nc.tensor.matmul(a_ps, w0c[:, d, :], hT[d][:, ssl],
                                         start=(d == 0), stop=(d == DT - 1))
                    for d in range(DT):
                        nc.tensor.matmul(b_ps, w1c[:, d, :], hT[d][:, ssl],
                                         start=(d == 0), stop=(d == DT - 1))
                    ga = gpool.tile([128, 512], BF16, tag="ga", name="ga")
                    nc.scalar.activation(out=ga, in_=a_ps, func=AF.Silu)
                    gb = gpool.tile([128, 512], BF16, tag="gb", name="gb")
                    nc.vector.tensor_copy(gb, b_ps)
                    nc.vector.tensor_tensor(out=gT[ft][:, ssl], in0=ga, in1=gb,
                                            op=mybir.AluOpType.mult)

        with ExitStack() as ph:
            wpool = ph.enter_context(tc.tile_pool(name="wogp", bufs=1))
            wog4 = [wog0] + [wpool.tile([128, 4, D], BF16, name=f"wog{g}")
                             for g in range(1, 4)]
            for g in range(1, 4):
                for f in range(4):
                    nc.sync.dma_start(out=wog4[g][:, f, :],
                                      in_=wog_p[:, 4 * g + f, :])
            opool = ph.enter_context(tc.tile_pool(name="outp", bufs=8))
            pso = ph.enter_context(tc.tile_pool(name="pso", bufs=1, space="PSUM"))
            # f-outer superblocks: 2 s-tiles x 4 d-chunks = 8 PSUM banks live,
            # each wog group is consumed progressively (DMA overlaps compute)
            for sb in range(ST // 2):
                o_ps = [pso.tile([128, 512], F32, tag=f"o{t}", name=f"o{t}")
                        for t in range(8)]
                for f in range(FT):
                    for i in range(2):
                        st = 2 * sb + i
                        ssl = slice(st * 128, (st + 1) * 128)
                        for dc in range(4):
                            nc.tensor.matmul(
                                o_ps[i * 4 + dc], gT[f][:, ssl],
                                wog4[f // 4][:, f % 4, dc * 512:(dc + 1) * 512],
                                start=(f == 0), stop=(f == FT - 1))
                for t in range(8):
                    st, dc = 2 * sb + t // 4, t % 4
                    osb = opool.tile([128, 512], F32, tag="osb", name="osb")
                    nc.vector.tensor_copy(osb, o_ps[t])
                    nc.sync.dma_start(
                        out=mlp[st * 128:(st + 1) * 128,
                                dc * 512:(dc + 1) * 512], in_=osb)
    legalize_waits(nc)
    return nc


# ---------------------------------------------------------------------------
# Persistent-jit SPMD runner
# ---------------------------------------------------------------------------
class SpmdRunner:
    def __init__(self, nc, n_cores=N_CORES):
        install_neuronx_cc_hook()
        self.nc = nc
        self.n_cores = n_cores
        partition_name = nc.partition_id_tensor.name if nc.partition_id_tensor else None
        in_names, out_names, out_avals, zero_outs = [], [], [], []
        for alloc in nc.m.functions[0].allocations:
            if not isinstance(alloc, mybir.MemoryLocationSet):
                continue
            name = alloc.memorylocations[0].name
            if alloc.kind == "ExternalInput":
                if name != partition_name:
                    in_names.append(name)
            elif alloc.kind == "ExternalOutput":
                shape = tuple(alloc.tensor_shape)
                dtype = mybir.dt.np(alloc.dtype)
                out_avals.append(jax.core.ShapedArray(shape, dtype))
                out_names.append(name)
                zero_outs.append(np.zeros(shape, dtype))
        self.in_names, self.out_names = in_names, out_names
        self.out_avals, self.zero_outs = out_avals, zero_outs
        n_params = len(in_names)
        n_outs = len(out_names)
        self.n_params = n_params
        all_in_names = list(in_names) + list(out_names)
        if partition_name is not None:
            all_in_names.append(partition_name)

        def _body(*args):
            operands = list(args)
            if partition_name is not None:
                operands.append(partition_id_tensor())
            outs = _bass_exec_p.bind(
                *operands,
                out_avals=tuple(out_avals),
                in_names=tuple(all_in_names),
                out_names=tuple(out_names),
                lowering_input_output_aliases=(),
                sim_require_finite=True,
                sim_require_nnan=True,
                nc=nc,
            )
            return tuple(outs)

        devices = jax.devices()[:n_cores]
        assert len(devices) >= n_cores, f"need {n_cores} neuron cores"
        self.mesh = Mesh(np.asarray(devices[:n_cores]), ("core",))
        in_specs = (PartitionSpec("core"),) * (n_params + n_outs)
        out_specs = (PartitionSpec("core"),) * n_outs
        donate = tuple(range(n_params, n_params + n_outs))
        self._fn = jax.jit(
            shard_map(_body, mesh=self.mesh, in_specs=in_specs,
                      out_specs=out_specs, check_rep=False),
            donate_argnums=donate, keep_unused=True)

        import jax.numpy as jnp
        shardings = tuple(NamedSharding(self.mesh, PartitionSpec("core"))
                          for _ in zero_outs)
        shapes = [(n_cores * z.shape[0], *z.shape[1:]) for z in zero_outs]
        dtypes = [z.dtype for z in zero_outs]

        def mk():
            return tuple(jnp.zeros(s, d) for s, d in zip(shapes, dtypes))

        self._zeros_fn = jax.jit(mk, out_shardings=shardings)

    def concat_inputs(self, in_maps):
        per_core = [[np.asarray(m[name]) for name in self.in_names] for m in in_maps]
        return [np.concatenate([per_core[c][i] for c in range(self.n_cores)], axis=0)
                for i in range(self.n_params)]

    def device_put_inputs(self, concat_in):
        sh = NamedSharding(self.mesh, PartitionSpec("core"))
        return [jax.device_put(a, sh) for a in concat_in]

    def __call__(self, concat_in):
        zeros = list(self._zeros_fn())
        return self._fn(*(list(concat_in) + zeros))

    def split_outputs(self, out_arrs):
        return [
            {name: np.asarray(out_arrs[i]).reshape(
                self.n_cores, *self.out_avals[i].shape)[c]
             for i, name in enumerate(self.out_names)}
            for c in range(self.n_cores)
        ]

    def run(self, in_maps):
        return self.split_outputs(self(self.concat_inputs(in_maps)))


_RUNNERS = {}


def _get_runners():
    if "attn" not in _RUNNERS:
        _RUNNERS["attn"] = SpmdRunner(build_attn())
        _RUNNERS["mlp"] = SpmdRunner(build_mlp())
    return _RUNNERS["attn"], _RUNNERS["mlp"]


# ---------------------------------------------------------------------------
# Host-side prep + the public kernel() entry point
# ---------------------------------------------------------------------------
def _attn_in_maps(x, ln1_scale, wq, wk, wv, qln_scale, kln_scale, wo):
    wq_f = wq * ln1_scale[:, None, None]
    wk_f = wk * ln1_scale[:, None, None]
    wv_f = wv * ln1_scale[:, None, None]
    qkw = (qln_scale * kln_scale).astype(np.float32)

    mask = np.zeros((128, 1024), np.float32)
    j = np.arange(1024)[None, :]
    k = np.arange(128)[:, None]
    mask[(j - 512) >= k] = 1.0
    # paired masks for the fused [128,1024] exp tiles: boundary kt offsets
    # pair A covers kt offsets (0,1) -> mask slices at 512, 384;
    # pair B covers kt offsets (2,3) -> mask slices at 256, 128.
    maska = np.concatenate([mask[:, 512:1024], mask[:, 384:896]], 1).astype(bf16)
    maskb = np.concatenate([mask[:, 256:768], mask[:, 128:640]], 1).astype(bf16)
    qw_tile = np.ascontiguousarray(
        np.broadcast_to(np.tile(qkw, NH)[None, :], (128, NH * HD))).astype(f16)

    in_maps = []
    for c in range(N_CORES):
        b, hg = c // 4, c % 4
        hs = slice(hg * NH, (hg + 1) * NH)
        xb = x[b]
        rvb = (1.0 / np.sqrt((xb.astype(np.float64) ** 2).mean(-1) + EPS)).astype(np.float32)
        in_maps.append({
            "xt": np.ascontiguousarray(xb.T).astype(f16),
            "rv": np.ascontiguousarray(rvb.reshape(ST, 128).T),
            "wq": np.ascontiguousarray(wq_f[:, hs].reshape(D, NH * HD)).astype(f16),
            "wk": np.ascontiguousarray(wk_f[:, hs].reshape(D, NH * HD)).astype(f16),
            "wv": np.ascontiguousarray(wv_f[:, hs].reshape(D, NH * HD)).astype(f16),
            "wo": np.ascontiguousarray(wo[hs].reshape(NH * HD, D)).astype(bf16),
            "qw": qw_tile,
            "maska": maska,
            "maskb": maskb,
        })
    return in_maps


def _mlp_in_maps(inter, ln2_scale, wi0, wi1, wout):
    wi0_f = (wi0 * ln2_scale[:, None]).astype(bf16)
    wi1_f = (wi1 * ln2_scale[:, None]).astype(bf16)
    wout_b = wout.astype(bf16)
    in_maps = []
    hts = []
    for b in range(B):
        ib = inter[b]
        r2 = (1.0 / np.sqrt((ib.astype(np.float64) ** 2).mean(-1) + EPS)).astype(np.float32)
        h = ib * r2[:, None]
        hts.append(np.ascontiguousarray(h.T).astype(bf16))
    for c in range(N_CORES):
        b, fg = c // 4, c % 4
        fsl = slice(fg * FL, (fg + 1) * FL)
        in_maps.append({
            "ht": hts[b],
            "wi0": np.ascontiguousarray(wi0_f[:, fsl]),
            "wi1": np.ascontiguousarray(wi1_f[:, fsl]),
            "wog": np.ascontiguousarray(wout_b[fsl, :]),
        })
    return in_maps


def kernel(x, ln1_scale, wq, wk, wv, qln_scale, kln_scale, wo, ln2_scale,
           wi0, wi1, wout):
    x = np.asarray(x, np.float32)
    ln1_scale = np.asarray(ln1_scale, np.float32)
    wq = np.asarray(wq, np.float32)
    wk = np.asarray(wk, np.float32)
    wv = np.asarray(wv, np.float32)
    qln_scale = np.asarray(qln_scale, np.float32)
    kln_scale = np.asarray(kln_scale, np.float32)
    wo = np.asarray(wo, np.float32)
    ln2_scale = np.asarray(ln2_scale, np.float32)
    wi0 = np.asarray(wi0, np.float32)
    wi1 = np.asarray(wi1, np.float32)
    wout = np.asarray(wout, np.float32)

    attn_runner, mlp_runner = _get_runners()

    a_maps = _attn_in_maps(x, ln1_scale, wq, wk, wv, qln_scale, kln_scale, wo)
    a_res = attn_runner.run(a_maps)
    attn_out = np.zeros((B, S, D), np.float32)
    for c in range(N_CORES):
        attn_out[c // 4] += a_res[c]["attn"]

    inter = x + attn_out

    m_maps = _mlp_in_maps(inter, ln2_scale, wi0, wi1, wout)
    m_res = mlp_runner.run(m_maps)
    out = inter.copy()
    for c in range(N_CORES):
        out[c // 4] += m_res[c]["mlp"]
    return out.astype(np.float32)



# revision 19
# speedup vs baseline: 1.0093x; 1.0093x over previous
"""Trainium2 Bass kernel for nn_DecoderLayer_70205535421363.

Decoder layer (pre-LN, T5-style RMSNorm, QK-norm attention + gated-silu MLP)
B=2, S=2048, D=2048, H=16, HD=128, F=8192, fp32.

Strategy: 8 cores = 2 batches x 4 shards, two SPMD launches.
  Launch A (attention): core c handles batch c//4, heads 4*(c%4)..+4.
    Q/K projections + scores in float32r (bf16-rate matmul, ~tf32 precision);
    unnormalized softmax (exp on ACT, denominators via a ones-column in the
    AV matmul); AV + output projection in bf16. Outputs per-core partial
    attention output; host sums the 4 head-shards per batch.
  Launch B (MLP): core c handles batch c//4, mlp columns 2048*(c%4)..+2048.
    Host computes inter = x + attn_out and pre-norms/transposes it; device
    does the three matmuls + silu gating in bf16, fp32 accumulation.
All learned norm scales are folded on the host (ln1/ln2 into the weights;
qln*kln applied to q_hat on device; rmsnorm r factors cancel for q/k and are
applied to v / folded into h on host).
"""
import os

import numpy as np
import ml_dtypes
from contextlib import ExitStack

import jax
from jax.sharding import Mesh, PartitionSpec, NamedSharding
from jax.experimental.shard_map import shard_map

import concourse.bass as bass
import concourse.tile as tile
import concourse.mybir as mybir
from concourse import bass2jax
from concourse.bass2jax import _bass_exec_p, install_neuronx_cc_hook, partition_id_tensor
from concourse.vector_clock import ScopedClock
from concourse.masks import make_identity

F32 = mybir.dt.float32
F32R = mybir.dt.float32r
F16 = mybir.dt.float16
BF16 = mybir.dt.bfloat16
AF = mybir.ActivationFunctionType
bf16 = ml_dtypes.bfloat16
f16 = np.float16

B, S, D, H, HD, F = 2, 2048, 2048, 16, 128, 8192
EPS = 1e-6
NH = 4            # heads per core
FL = F // 4       # mlp columns per core
ST = S // 128
DT = D // 128
FT = FL // 128
N_CORES = 8

MAX_WAITS = 1     # this walrus build allows one sync-wait per instruction


# ---------------------------------------------------------------------------
# Tile workarounds for the 1-sync-wait-per-instruction walrus limit
# ---------------------------------------------------------------------------
class TileContextFixed(tile.TileContext):
    def _drain_and_barrier(self, tick_clock, wait_clock):
        nc = self.nc
        probe = nc.sync.nop(nofuse=True)
        wait_clock.add_sem_waits(probe.ins, ScopedClock({None: tick_clock.global_clock}))
        si = probe.ins.sync_info
        waits = list(si.on_wait) if si is not None else []
        if len(waits) > MAX_WAITS:
            si.on_wait = waits[:MAX_WAITS]
            rest = waits[MAX_WAITS:]
            for i in range(0, len(rest), MAX_WAITS):
                extra = nc.sync.nop(nofuse=True)
                extra.ins.sync_info = mybir.SyncInfo(
                    on_wait=rest[i:i + MAX_WAITS], on_update=[])
        nc.sync.drain()
        nc.all_engine_barrier()
        assert self.sems is not None
        popped = nc._tile_sem_poison_stack.pop()
        assert popped is self._sem_poison
        nc.clear_and_free_semaphores(list(self.sems.allocated().values()))
        nc.all_engine_barrier()


def legalize_waits(nc, max_waits=MAX_WAITS):
    for fn in nc.m.functions:
        for bb in fn.blocks:
            insts = bb.instructions
            new_insts = []
            changed = False
            for inst in insts:
                si = inst.sync_info
                if si is not None and len(si.on_wait) > max_waits:
                    waits = list(si.on_wait)
                    keep = waits[:max_waits]
                    rest = waits[max_waits:]
                    for i in range(0, len(rest), max_waits):
                        nop = mybir.InstNoOp(
                            name=nc.get_next_instruction_name(),
                            engine=inst.engine, ins=[], outs=[])
                        nop.sync_info = mybir.SyncInfo(
                            on_wait=rest[i:i + max_waits], on_update=[])
                        nc.register_instruction(nop)
                        new_insts.append(nop)
                        changed = True
                    si.on_wait = keep
                new_insts.append(inst)
            if changed:
                insts.clear()
                insts.extend(new_insts)


# ---------------------------------------------------------------------------
# Launch A: attention partial (one batch, NH heads per core)
# ---------------------------------------------------------------------------
def build_attn(reps=1):
    nc = bass.Bass()
    xt = nc.dram_tensor("xt", [D, S], F16, kind="ExternalInput")
    rv = nc.dram_tensor("rv", [128, ST], F32, kind="ExternalInput")
    wq = nc.dram_tensor("wq", [D, NH * HD], F16, kind="ExternalInput")
    wk = nc.dram_tensor("wk", [D, NH * HD], F16, kind="ExternalInput")
    wv = nc.dram_tensor("wv", [D, NH * HD], F16, kind="ExternalInput")
    wo = nc.dram_tensor("wo", [NH * HD, D], BF16, kind="ExternalInput")
    qw = nc.dram_tensor("qw", [128, NH * HD], F16, kind="ExternalInput")
    maska = nc.dram_tensor("maska", [128, 1024], BF16, kind="ExternalInput")
    maskb = nc.dram_tensor("maskb", [128, 1024], BF16, kind="ExternalInput")
    attn = nc.dram_tensor("attn", [S, D], F32, kind="ExternalOutput")

    # p-major views for single-DMA loads
    xt_p = xt.rearrange("(dt p) s -> p dt s", p=128)
    wq_p = wq.rearrange("(dt p) f -> p dt f", p=128)
    wk_p = wk.rearrange("(dt p) f -> p dt f", p=128)
    wv_bp = wv.rearrange("(dt p) f -> p dt f", p=128)
    wo_p = wo.rearrange("(h p) d -> p h d", p=128)

    with TileContextFixed(nc) as tc:
      for _rep in range(reps):
       with ExitStack() as top:
        consts = top.enter_context(tc.tile_pool(name="consts", bufs=1))
        qw_sb = consts.tile([128, NH * HD], F16, name="qw_sb")
        maska_sb = consts.tile([128, 1024], BF16, name="maska_sb")
        maskb_sb = consts.tile([128, 1024], BF16, name="maskb_sb")
        rv_sb = consts.tile([128, ST], F32, name="rv_sb")
        eps_sb = consts.tile([128, 1], F32, name="eps_sb")
        nc.vector.memset(eps_sb, EPS)
        id_h = consts.tile([128, 128], F16, name="id_h")
        make_identity(nc, id_h)
        id_b = consts.tile([128, 128], BF16, name="id_b")
        make_identity(nc, id_b)
        # wo prefetched at top level so phase 2 starts without a DMA stall
        wopool = top.enter_context(tc.tile_pool(name="wop", bufs=1))
        wo_sb = wopool.tile([128, NH, D], BF16, name="wo_sb")

        persist = top.enter_context(tc.tile_pool(name="persist", bufs=1))
        QT = [persist.tile([128, S], F16, tag=f"qt{h}", name=f"qt{h}")
              for h in range(NH)]
        KT = [persist.tile([128, S], F16, tag=f"kt{h}", name=f"kt{h}")
              for h in range(NH)]
        VA = [persist.tile([128, ST, 132], BF16, tag=f"va{h}", name=f"va{h}")
              for h in range(NH)]
        for h in range(NH):
            nc.vector.memset(VA[h][:, :, 128:129], 1.0)

        # ---- phase 1a: Q, K projections + per-head rmsnorm + transpose ----
        with ExitStack() as ph:
            wpool = ph.enter_context(tc.tile_pool(name="wqk", bufs=1))
            xpool = ph.enter_context(tc.tile_pool(name="xcol", bufs=2))
            wq_sb = [wpool.tile([128, 4, NH * HD], F16, name=f"wq_g{g}")
                     for g in range(4)]
            wk_sb = [wpool.tile([128, 4, NH * HD], F16, name=f"wk_g{g}")
                     for g in range(4)]
            wv_bf = [wpool.tile([128, 4, NH * HD], F16, name=f"wv_g{g}")
                     for g in range(4)]
            xq = []

            def load_x(st):
                xcol = xpool.tile([128, DT, 128], F16, tag="x", name="xcol")
                nc.sync.dma_start(out=xcol,
                                  in_=xt_p[:, :, st * 128:(st + 1) * 128])
                xq.append(xcol)

            # x strip + wq first (first matmuls depend only on these), then
            # wk/wv, then the small consts and the phase-2 wo prefetch.
            load_x(0)
            for g in range(4):
                nc.sync.dma_start(out=wq_sb[g], in_=wq_p[:, 4 * g:4 * g + 4, :])
                if g == 0:
                    load_x(1)
            for g in range(4):
                nc.sync.dma_start(out=wk_sb[g], in_=wk_p[:, 4 * g:4 * g + 4, :])
                nc.sync.dma_start(out=wv_bf[g], in_=wv_bp[:, 4 * g:4 * g + 4, :])
            nc.sync.dma_start(out=rv_sb, in_=rv[:, :])
            nc.sync.dma_start(out=qw_sb, in_=qw[:, :])
            nc.sync.dma_start(out=maska_sb, in_=maska[:, :])
            nc.sync.dma_start(out=maskb_sb, in_=maskb[:, :])
            for h in range(NH):
                nc.sync.dma_start(out=wo_sb[:, h, :], in_=wo_p[:, h, :])
            tpool = ph.enter_context(tc.tile_pool(name="p1tmp", bufs=2))
            spool = ph.enter_context(tc.tile_pool(name="p1sc", bufs=3))
            psq = ph.enter_context(tc.tile_pool(name="psq", bufs=2, space="PSUM"))
            pst = ph.enter_context(tc.tile_pool(name="pst", bufs=1, space="PSUM"))

            def p1_mm_norm(st):
                if st + 2 < ST:
                    load_x(st + 2)
                xcol = xq.pop(0)
                q_ps = psq.tile([128, NH * HD], F32, tag="q", name="q_ps")
                k_ps = psq.tile([128, NH * HD], F32, tag="k", name="k_ps")
                for d in range(DT):
                    nc.tensor.matmul(q_ps, xcol[:, d, :], wq_sb[d // 4][:, d % 4, :],
                                     start=(d == 0), stop=(d == DT - 1))
                for d in range(DT):
                    nc.tensor.matmul(k_ps, xcol[:, d, :], wk_sb[d // 4][:, d % 4, :],
                                     start=(d == 0), stop=(d == DT - 1))
                v_ps = psq.tile([128, NH * HD], F32, tag="v", name="v_ps")
                for d in range(DT):
                    nc.tensor.matmul(v_ps, xcol[:, d, :], wv_bf[d // 4][:, d % 4, :],
                                     start=(d == 0), stop=(d == DT - 1))
                for h in range(NH):
                    sl = slice(h * HD, (h + 1) * HD)
                    nc.scalar.activation(out=VA[h][:, st, 0:128], in_=v_ps[:, sl],
                                         func=AF.Copy, scale=rv_sb[:, st:st + 1])
                sq = spool.tile([128, HD], F32, tag="sq", name="sq")
                ssq = spool.tile([128, NH], F32, tag="ssq", name="ssq")
                ssk = spool.tile([128, NH], F32, tag="ssk", name="ssk")
                for h in range(NH):
                    sl = slice(h * HD, (h + 1) * HD)
                    nc.scalar.activation(out=sq, in_=q_ps[:, sl],
                                         func=AF.Square, accum_out=ssq[:, h:h + 1])
                for h in range(NH):
                    sl = slice(h * HD, (h + 1) * HD)
                    nc.scalar.activation(out=sq, in_=k_ps[:, sl],
                                         func=AF.Square, accum_out=ssk[:, h:h + 1])
                rq = spool.tile([128, NH], F32, tag="rq", name="rq")
                nc.scalar.activation(out=rq, in_=ssq, func=AF.Sqrt,
                                     scale=1.0 / HD, bias=eps_sb)
                nc.vector.reciprocal(rq, rq)
                rk = spool.tile([128, NH], F32, tag="rk", name="rk")
                nc.scalar.activation(out=rk, in_=ssk, func=AF.Sqrt,
                                     scale=1.0 / HD, bias=eps_sb)
                nc.vector.reciprocal(rk, rk)
                qh = tpool.tile([128, NH * HD], F16, tag="qh", name="qh")
                for h in range(NH):
                    sl = slice(h * HD, (h + 1) * HD)
                    nc.vector.tensor_scalar_mul(qh[:, sl], q_ps[:, sl], rq[:, h:h + 1])
                nc.vector.tensor_tensor(out=qh, in0=qh, in1=qw_sb,
                                        op=mybir.AluOpType.mult)
                ksb = tpool.tile([128, NH * HD], F16, tag="ksb", name="ksb")
                for h in range(NH):
                    sl = slice(h * HD, (h + 1) * HD)
                    nc.vector.tensor_scalar_mul(ksb[:, sl], k_ps[:, sl], rk[:, h:h + 1])
                return qh, ksb

            def p1_transpose(st, qh, ksb):
                qtk = pst.tile([128, 2 * NH * HD], F16, tag="qtk", name="qtk")
                qt_ps = qtk[:, 0:NH * HD]
                kt_ps = qtk[:, NH * HD:2 * NH * HD]
                for h in range(NH):
                    sl = slice(h * HD, (h + 1) * HD)
                    nc.tensor.transpose(qt_ps[:, sl], qh[:, sl], id_h)
                    nc.tensor.transpose(kt_ps[:, sl], ksb[:, sl], id_h)
                ssl = slice(st * 128, (st + 1) * 128)
                for h in range(NH):
                    sl = slice(h * HD, (h + 1) * HD)
                    nc.vector.tensor_copy(QT[h][:, ssl], qt_ps[:, sl])
                    nc.vector.tensor_copy(KT[h][:, ssl], kt_ps[:, sl])

            prev = None
            for st in range(ST):
                cur = p1_mm_norm(st)
                if prev is not None:
                    p1_transpose(prev[0], *prev[1])
                prev = (st, cur)
            p1_transpose(prev[0], *prev[1])

        # ---- phase 2+3: attention + output projection, qc-outer ----
        with ExitStack() as ph:
            ppool = ph.enter_context(tc.tile_pool(name="probs", bufs=6))
            fpool = ph.enter_context(tc.tile_pool(name="fin", bufs=4))
            apool = ph.enter_context(tc.tile_pool(name="attnT", bufs=2))
            opool = ph.enter_context(tc.tile_pool(name="outp", bufs=4))
            pslg = ph.enter_context(tc.tile_pool(name="pslg", bufs=2, space="PSUM"))
            psav = ph.enter_context(tc.tile_pool(name="psav", bufs=1, space="PSUM"))
            psat = ph.enter_context(tc.tile_pool(name="psat", bufs=1, space="PSUM"))
            psot = ph.enter_context(tc.tile_pool(name="psot", bufs=1, space="PSUM"))

            def attend_head(h, qc, attnT_h):
                q0 = qc * 512
                att2 = [psav.tile([128, 2, 132], F32, tag=f"att{i}",
                                  name=f"att{i}") for i in range(2)]
                att_ps = [att2[qs // 2][:, qs % 2, :] for qs in range(4)]
                for p in range(2 * qc + 2):
                    lg = pslg.tile([128, 1024], F32, tag="lg", name="lg")
                    for half in range(2):
                        kt = 2 * p + half
                        nc.tensor.matmul(
                            lg[:, half * 512:(half + 1) * 512],
                            KT[h][:, kt * 128:(kt + 1) * 128],
                            QT[h][:, q0:q0 + 512], start=True, stop=True)
                    pr = ppool.tile([128, 1024], BF16, tag="pr", name="pr")
                    nc.scalar.activation(out=pr, in_=lg, func=AF.Exp)
                    if 2 * p >= qc * 4:
                        msk = maska_sb if 2 * p == qc * 4 else maskb_sb
                        nc.vector.tensor_tensor(
                            out=pr, in0=pr, in1=msk, op=mybir.AluOpType.mult)
                    for half in range(2):
                        kt = 2 * p + half
                        for qs in range(4):
                            qt = qc * 4 + qs
                            if kt <= qt:
                                # two accumulators share a PSUM bank; start=True
                                # clears has_written bank-wide, so only the first
                                # group in each bank may use it — the second
                                # group's kt=0 write lands on cleared bits and
                                # overwrites rather than accumulates.
                                nc.tensor.matmul(
                                    att_ps[qs][:, 0:129],
                                    pr[:, half * 512 + qs * 128:
                                       half * 512 + (qs + 1) * 128],
                                    VA[h][:, kt, 0:129],
                                    start=(kt == 0 and qs % 2 == 0),
                                    stop=(kt == qt), skip_group_check=True)
                return att_ps

            def finalize_head(qc, att_ps, attnT_h):
                for qs in range(4):
                    dr = fpool.tile([128, 1], F32, tag="dr", name="dr")
                    nc.vector.reciprocal(dr, att_ps[qs][:, 128:129])
                    asb = fpool.tile([128, 128], BF16, tag="asb", name="asb")
                    nc.vector.tensor_scalar_mul(asb, att_ps[qs][:, 0:128], dr)
                    at_ps = psat.tile([128, 128], BF16, tag="atp", name="at_ps")
                    nc.tensor.transpose(at_ps, asb, id_b)
                    nc.scalar.activation(
                        out=attnT_h[:, qs * 128:(qs + 1) * 128], in_=at_ps,
                        func=AF.Copy)

            def emit_part(attnT, qc, qs):
                # one q-subtile of the output projection; parts are issued
                # right AFTER each finalize_head so the finalize's DVE ops
                # (which release the shared AV accumulators) are never queued
                # behind this part's copy burst on the strict-FIFO DVE.
                qt = qc * 4 + qs
                for dc in range(D // 512):
                    o_ps = psot.tile([128, 512], F32, tag="o", name="o_ps")
                    for h in range(NH):
                        nc.tensor.matmul(
                            o_ps, attnT[h][:, qs * 128:(qs + 1) * 128],
                            wo_sb[:, h, dc * 512:(dc + 1) * 512],
                            start=(h == 0), stop=(h == NH - 1))
                    osb = opool.tile([128, 512], F32, tag="osb", name="osb")
                    nc.vector.tensor_copy(osb, o_ps)
                    nc.sync.dma_start(
                        out=attn[qt * 128:(qt + 1) * 128,
                                 dc * 512:(dc + 1) * 512],
                        in_=osb)

            pending = None
            for qc in range(S // 512):
                attnT = [apool.tile([128, 512], BF16, tag=f"at{h}", name=f"at{h}")
                         for h in range(NH)]
                prev = None
                for h in range(NH):
                    att_ps = attend_head(h, qc, attnT[h])
                    if prev is not None:
                        finalize_head(qc, prev[1], attnT[prev[0]])
                        if pending is not None:
                            emit_part(pending[0], pending[1], prev[0])
                    prev = (h, att_ps)
                finalize_head(qc, prev[1], attnT[prev[0]])
                if pending is not None:
                    emit_part(pending[0], pending[1], 3)
                pending = (attnT, qc)
            for qs in range(4):
                emit_part(pending[0], pending[1], qs)
    legalize_waits(nc)
    return nc


# ---------------------------------------------------------------------------
# Launch B: gated MLP partial (one batch, FL mlp columns per core)
# ---------------------------------------------------------------------------
def build_mlp(reps=1):
    nc = bass.Bass()
    ht = nc.dram_tensor("ht", [D, S], BF16, kind="ExternalInput")
    wi0 = nc.dram_tensor("wi0", [D, FL], BF16, kind="ExternalInput")
    wi1 = nc.dram_tensor("wi1", [D, FL], BF16, kind="ExternalInput")
    wog = nc.dram_tensor("wog", [FL, D], BF16, kind="ExternalInput")
    mlp = nc.dram_tensor("mlp", [S, D], F32, kind="ExternalOutput")

    wi0_p = wi0.rearrange("(dt p) f -> p dt f", p=128)
    wi1_p = wi1.rearrange("(dt p) f -> p dt f", p=128)
    ht_p = ht.rearrange("(dt p) s -> p dt s", p=128)
    wog_p = wog.rearrange("(ft p) d -> p ft d", p=128)

    with TileContextFixed(nc) as tc:
      for _rep in range(reps):
       with ExitStack() as top:
        persist = top.enter_context(tc.tile_pool(name="persist", bufs=1))
        gT = [persist.tile([128, S], BF16, tag=f"g{f}", name=f"g{f}")
              for f in range(FT)]
        # first wog group prefetched during phase 1 so phase 2 starts hot
        wog0 = persist.tile([128, 4, D], BF16, tag="wog0", name="wog0")

        with ExitStack() as ph:
            hpool = ph.enter_context(tc.tile_pool(name="htp", bufs=1))
            wpool = ph.enter_context(tc.tile_pool(name="wcol", bufs=3))
            gpool = ph.enter_context(tc.tile_pool(name="gtmp", bufs=4))
            psab = ph.enter_context(tc.tile_pool(name="psab", bufs=2, space="PSUM"))

            def load_w(ft):
                fsl = slice(ft * 128, (ft + 1) * 128)
                w0c = wpool.tile([128, DT, 128], BF16, tag="w0", name="w0c")
                w1c = wpool.tile([128, DT, 128], BF16, tag="w1", name="w1c")
                nc.sync.dma_start(out=w0c, in_=wi0_p[:, :, fsl])
                nc.sync.dma_start(out=w1c, in_=wi1_p[:, :, fsl])
                return w0c, w1c

            wq_pending = [load_w(0)]
            hT = [hpool.tile([128, S], BF16, tag=f"h{d}", name=f"h{d}")
                  for d in range(DT)]
            # first s-wave of hT lands first so matmuls start early
            for d in range(DT):
                nc.sync.dma_start(out=hT[d][:, 0:512], in_=ht_p[:, d, 0:512])
            for d in range(DT):
                nc.sync.dma_start(out=hT[d][:, 512:S], in_=ht_p[:, d, 512:S])
            for f in range(4):
                nc.sync.dma_start(out=wog0[:, f, :], in_=wog_p[:, f, :])
            for ft in range(FT):
                if ft + 1 < FT:
                    wq_pending.append(load_w(ft + 1))
                w0c, w1c = wq_pending.pop(0)
                for sc in range(S // 512):
                    ssl = slice(sc * 512, (sc + 1) * 512)
                    a_ps = psab.tile([128, 512], F32, tag="a", name="a_ps")
                    b_ps = psab.tile([128, 512], F32, tag="b", name="b_ps")
                    for d in range(DT):
                        nc.tensor.matmul(a_ps, w0c[:, d, :], hT[d][:, ssl],
                                         start=(d == 0), stop=(d == DT - 1))
                    for d in range(DT):
                        nc.tensor.matmul(b_ps, w1c[:, d, :], hT[d][:, ssl],
                                         start=(d == 0), stop=(d == DT - 1))
                    ga = gpool.tile([128, 512], BF16, tag="ga", name="ga")
                    nc.scalar.activation(out=ga, in_=a_ps, func=AF.Silu)
                    gb = gpool.tile([128, 512], BF16, tag="gb", name="gb")
                    nc.vector.tensor_copy(gb, b_ps)
                    nc.vector.tensor_tensor(out=gT[ft][:, ssl], in0=ga, in1=gb,
                                            op=mybir.AluOpType.mult)

        with ExitStack() as ph:
            wpool = ph.enter_context(tc.tile_pool(name="wogp", bufs=1))
            wog4 = [wog0] + [wpool.tile([128, 4, D], BF16, name=f"wog{g}")
                             for g in range(1, 4)]
            for g in range(1, 4):
                for f in range(4):
                    nc.sync.dma_start(out=wog4[g][:, f, :],
                                      in_=wog_p[:, 4 * g + f, :])
            opool = ph.enter_context(tc.tile_pool(name="outp", bufs=8))
            pso = ph.enter_context(tc.tile_pool(name="pso", bufs=1, space="PSUM"))
            # f-outer superblocks: 2 s-tiles x 4 d-chunks = 8 PSUM banks live,
            # each wog group is consumed progressively (DMA overlaps compute)
            for sb in range(ST // 2):
                o_ps = [pso.tile([128, 512], F32, tag=f"o{t}", name=f"o{t}")
                        for t in range(8)]
                for f in range(FT):
                    for i in range(2):
                        st = 2 * sb + i
                        ssl = slice(st * 128, (st + 1) * 128)
                        for dc in range(4):
                            nc.tensor.matmul(
                                o_ps[i * 4 + dc], gT[f][:, ssl],
                                wog4[f // 4][:, f % 4, dc * 512:(dc + 1) * 512],
                                start=(f == 0), stop=(f == FT - 1))
                for t in range(8):
                    st, dc = 2 * sb + t // 4, t % 4
                    osb = opool.tile([128, 512], F32, tag="osb", name="osb")
                    nc.vector.tensor_copy(osb, o_ps[t])
                    nc.sync.dma_start(
                        out=mlp[st * 128:(st + 1) * 128,
                                dc * 512:(dc + 1) * 512], in_=osb)
    legalize_waits(nc)
    return nc


# ---------------------------------------------------------------------------
# Persistent-jit SPMD runner
# ---------------------------------------------------------------------------
class SpmdRunner:
    def __init__(self, nc, n_cores=N_CORES):
        install_neuronx_cc_hook()
        self.nc = nc
        self.n_cores = n_cores
        partition_name = nc.partition_id_tensor.name if nc.partition_id_tensor else None
        in_names, out_names, out_avals, zero_outs = [], [], [], []
        for alloc in nc.m.functions[0].allocations:
            if not isinstance(alloc, mybir.MemoryLocationSet):
                continue
            name = alloc.memorylocations[0].name
            if alloc.kind == "ExternalInput":
                if name != partition_name:
                    in_names.append(name)
            elif alloc.kind == "ExternalOutput":
                shape = tuple(alloc.tensor_shape)
                dtype = mybir.dt.np(alloc.dtype)
                out_avals.append(jax.core.ShapedArray(shape, dtype))
                out_names.append(name)
                zero_outs.append(np.zeros(shape, dtype))
        self.in_names, self.out_names = in_names, out_names
        self.out_avals, self.zero_outs = out_avals, zero_outs
        n_params = len(in_names)
        n_outs = len(out_names)
        self.n_params = n_params
        all_in_names = list(in_names) + list(out_names)
        if partition_name is not None:
            all_in_names.append(partition_name)

        def _body(*args):
            operands = list(args)
            if partition_name is not None:
                operands.append(partition_id_tensor())
            outs = _bass_exec_p.bind(
                *operands,
                out_avals=tuple(out_avals),
                in_names=tuple(all_in_names),
                out_names=tuple(out_names),
                lowering_input_output_aliases=(),
                sim_require_finite=True,
                sim_require_nnan=True,
                nc=nc,
            )
            return tuple(outs)

        devices = jax.devices()[:n_cores]
        assert len(devices) >= n_cores, f"need {n_cores} neuron cores"
        self.mesh = Mesh(np.asarray(devices[:n_cores]), ("core",))
        in_specs = (PartitionSpec("core"),) * (n_params + n_outs)
        out_specs = (PartitionSpec("core"),) * n_outs
        donate = tuple(range(n_params, n_params + n_outs))
        self._fn = jax.jit(
            shard_map(_body, mesh=self.mesh, in_specs=in_specs,
                      out_specs=out_specs, check_rep=False),
            donate_argnums=donate, keep_unused=True)

        import jax.numpy as jnp
        shardings = tuple(NamedSharding(self.mesh, PartitionSpec("core"))
                          for _ in zero_outs)
        shapes = [(n_cores * z.shape[0], *z.shape[1:]) for z in zero_outs]
        dtypes = [z.dtype for z in zero_outs]

        def mk():
            return tuple(jnp.zeros(s, d) for s, d in zip(shapes, dtypes))

        self._zeros_fn = jax.jit(mk, out_shardings=shardings)

    def concat_inputs(self, in_maps):
        per_core = [[np.asarray(m[name]) for name in self.in_names] for m in in_maps]
        return [np.concatenate([per_core[c][i] for c in range(self.n_cores)], axis=0)
                for i in range(self.n_params)]

    def device_put_inputs(self, concat_in):
        sh = NamedSharding(self.mesh, PartitionSpec("core"))
        return [jax.device_put(a, sh) for a in concat_in]

    def __call__(self, concat_in):
        zeros = list(self._zeros_fn())
        return self._fn(*(list(concat_in) + zeros))

    def split_outputs(self, out_arrs):
        return [
            {name: np.asarray(out_arrs[i]).reshape(
                self.n_cores, *self.out_avals[i].shape)[c]
             for i, name in enumerate(self.out_names)}
            for c in range(self.n_cores)
        ]

    def run(self, in_maps):
        return self.split_outputs(self(self.concat_inputs(in_maps)))


_RUNNERS = {}


def _get_runners():
    if "attn" not in _RUNNERS:
        _RUNNERS["attn"] = SpmdRunner(build_attn())
        _RUNNERS["mlp"] = SpmdRunner(build_mlp())
    return _RUNNERS["attn"], _RUNNERS["mlp"]


# ---------------------------------------------------------------------------
# Host-side prep + the public kernel() entry point
# ---------------------------------------------------------------------------
def _attn_in_maps(x, ln1_scale, wq, wk, wv, qln_scale, kln_scale, wo):
    wq_f = wq * ln1_scale[:, None, None]
    wk_f = wk * ln1_scale[:, None, None]
    wv_f = wv * ln1_scale[:, None, None]
    qkw = (qln_scale * kln_scale).astype(np.float32)

    mask = np.zeros((128, 1024), np.float32)
    j = np.arange(1024)[None, :]
    k = np.arange(128)[:, None]
    mask[(j - 512) >= k] = 1.0
    # paired masks for the fused [128,1024] exp tiles: boundary kt offsets
    # pair A covers kt offsets (0,1) -> mask slices at 512, 384;
    # pair B covers kt offsets (2,3) -> mask slices at 256, 128.
    maska = np.concatenate([mask[:, 512:1024], mask[:, 384:896]], 1).astype(bf16)
    maskb = np.concatenate([mask[:, 256:768], mask[:, 128:640]], 1).astype(bf16)
    qw_tile = np.ascontiguousarray(
        np.broadcast_to(np.tile(qkw, NH)[None, :], (128, NH * HD))).astype(f16)

    in_maps = []
    for c in range(N_CORES):
        b, hg = c // 4, c % 4
        hs = slice(hg * NH, (hg + 1) * NH)
        xb = x[b]
        rvb = (1.0 / np.sqrt((xb.astype(np.float64) ** 2).mean(-1) + EPS)).astype(np.float32)
        in_maps.append({
            "xt": np.ascontiguousarray(xb.T).astype(f16),
            "rv": np.ascontiguousarray(rvb.reshape(ST, 128).T),
            "wq": np.ascontiguousarray(wq_f[:, hs].reshape(D, NH * HD)).astype(f16),
            "wk": np.ascontiguousarray(wk_f[:, hs].reshape(D, NH * HD)).astype(f16),
            "wv": np.ascontiguousarray(wv_f[:, hs].reshape(D, NH * HD)).astype(f16),
            "wo": np.ascontiguousarray(wo[hs].reshape(NH * HD, D)).astype(bf16),
            "qw": qw_tile,
            "maska": maska,
            "maskb": maskb,
        })
    return in_maps


def _mlp_in_maps(inter, ln2_scale, wi0, wi1, wout):
    wi0_f = (wi0 * ln2_scale[:, None]).astype(bf16)
    wi1_f = (wi1 * ln2_scale[:, None]).astype(bf16)
    wout_b = wout.astype(bf16)
    in_maps = []
    hts = []
    for b in range(B):
        ib = inter[b]
        r2 = (1.0 / np.sqrt((ib.astype(np.float64) ** 2).mean(-1) + EPS)).astype(np.float32)
        h = ib * r2[:, None]
        hts.append(np.ascontiguousarray(h.T).astype(bf16))
    for c in range(N_CORES):
        b, fg = c // 4, c % 4
        fsl = slice(fg * FL, (fg + 1) * FL)
        in_maps.append({
            "ht": hts[b],
            "wi0": np.ascontiguousarray(wi0_f[:, fsl]),
            "wi1": np.ascontiguousarray(wi1_f[:, fsl]),
            "wog": np.ascontiguousarray(wout_b[fsl, :]),
        })
    return in_maps


def kernel(x, ln1_scale, wq, wk, wv, qln_scale, kln_scale, wo, ln2_scale,
           wi0, wi1, wout):
    x = np.asarray(x, np.float32)
    ln1_scale = np.asarray(ln1_scale, np.float32)
    wq = np.asarray(wq, np.float32)
    wk = np.asarray(wk, np.float32)
    wv = np.asarray(wv, np.float32)
    qln_scale = np.asarray(qln_scale, np.float32)
    kln_scale = np.asarray(kln_scale, np.float32)
    wo = np.asarray(wo, np.float32)
    ln2_scale = np.asarray(ln2_scale, np.float32)
    wi0 = np.asarray(wi0, np.float32)
    wi1 = np.asarray(wi1, np.float32)
    wout = np.asarray(wout, np.float32)

    attn_runner, mlp_runner = _get_runners()

    a_maps = _attn_in_maps(x, ln1_scale, wq, wk, wv, qln_scale, kln_scale, wo)
    a_res = attn_runner.run(a_maps)
    attn_out = np.zeros((B, S, D), np.float32)
    for c in range(N_CORES):
        attn_out[c // 4] += a_res[c]["attn"]

    inter = x + attn_out

    m_maps = _mlp_in_maps(inter, ln2_scale, wi0, wi1, wout)
    m_res = mlp_runner.run(m_maps)
    out = inter.copy()
    for c in range(N_CORES):
        out[c // 4] += m_res[c]["mlp"]
    return out.astype(np.float32)

